# revision 12
# baseline (speedup 1.0000x reference)
"""EvolveGCN-O forward pass on 8 Trainium2 NeuronCores (Bass/Tile).

Math (reference):
    w_new = LSTM-evolve(weight; w_ih, b_ih+b_hh)          # [C, C]
    out   = D^-1/2 (A + I) D^-1/2  X  w_new               # [N, C]

v2 strategy (per sharding hint: edges + scatter targets sharded):
  * Factor the norm: with y = dinv * x,  out = diag(dinv) (A+I) y W.
    The per-edge norm multiply disappears; y rows are prescaled on the
    host and stored in BF16 (halves gather bytes, enables full-rate
    bf16 PE matmuls; tolerance is 2e-2, bf16 keeps us ~1e-3).
  * Destination nodes padded to NPAD (multiple of 128*8); 128-node
    blocks; each core owns nbc consecutive blocks, processed in chunks
    of 7 (7 PSUM banks accumulate 7 blocks; the 8th bank is scratch).
  * Self-loop term y[i]: contiguous rows DMA'd and transposed into the
    block's PSUM accumulator via a bf16 identity matmul (start=True).
  * Edges: host sorts by dst block and splits by source range (the
    dma_gather index is a SIGNED int16 offset from the call's base row,
    so one call reaches a 65536-row window -> 2 ranges cover N=100k).
    Every (block, range) segment is padded to a uniform tile count;
    padding slots carry dstl=255 so their one-hot column is all-zero
    (>=1 slack slot so no call ends on a negative index, which the
    ucode would drop).
  * Per edge tile of 128: gpsimd.dma_gather stages bf16 rows y[src]
    (one call per CALL_T tiles); the one-hot dst selectors for a whole
    block's tiles are built in ONE DVE is_equal via broadcast APs; PE
    accumulates aggT += tile^T-routed sums. Per block:
    Y = (aggT^T @ w_new) * dinv[dst], DMA out.
  * w_new computed on-device (3 matmuls + activations), redundantly per
    core. No collectives: block ownership makes outputs disjoint.
"""
import sys

for _p in ("/opt/trn_rl_repo", "/root/.axon_site/_ro/trn_rl_repo"):
    if _p not in sys.path:
        sys.path.append(_p)

import numpy as np

N, C, E = 100000, 128, 1600000  # problem shape (hardcoded per spec)
P = 128
N_CORES = 8
CHUNK = 7  # blocks per PSUM-resident chunk
IDX_WIN = 32768  # int16 signed reach below/above base
CALL_T = 8  # edge tiles per dma_gather call (1024 idx = HW per-call max)


def _cdiv(a, b):
    return -(-a // b)


def prep_inputs(x, edge_index, weight, w_ih, b_ih, b_hh, n=N):
    """Host-side sharding/index prep.

    Returns (in_maps, meta) where meta = (t_r tuple, nbc, chunk_sizes).
    """
    import ml_dtypes

    bf16 = ml_dtypes.bfloat16
    x = np.ascontiguousarray(np.asarray(x, dtype=np.float32))
    ei = np.asarray(edge_index)
    src_e = ei[0].astype(np.int64)
    dst_e = ei[1].astype(np.int64)

    npad = _cdiv(n, P * N_CORES) * P * N_CORES
    nb = npad // P
    nbc = nb // N_CORES

    # degrees include self loops
    deg = (np.bincount(dst_e, minlength=n) + 1).astype(np.float32)
    dinv = (1.0 / np.sqrt(deg)).astype(np.float32)
    d1 = np.zeros(npad, np.float32)
    d1[:n] = dinv

    # prescaled node rows y = dinv * x, bf16
    ypad = np.zeros((npad, C), bf16)
    ypad[:n] = (x * dinv[:, None]).astype(bf16)

    # source ranges
    split = npad // 2
    bases = (max(0, split - IDX_WIN), max(0, npad - IDX_WIN))
    rng_of = (src_e >= split).astype(np.int64)

    # sort edges by (block, range) then pack
    blk = dst_e >> 7
    order = np.argsort(blk * 2 + rng_of, kind="stable")
    srcs = src_e[order]
    dsts = dst_e[order]
    rngs = rng_of[order]

    # per-(block, range) counts -> uniform tile counts. +4 slack slots so
    # every cell keeps spare padding for the call-final swap below.
    cell = blk[order] * 2 + rngs
    counts = np.bincount(cell, minlength=nb * 2).reshape(nb, 2)
    t_r = tuple(int(_cdiv(int(counts[:, r].max()) + 4, P)) for r in range(2))
    cap = (t_r[0] * P, t_r[1] * P)

    # chunk structure (uniform across cores)
    chunk_sizes = [min(CHUNK, nbc - i) for i in range(0, nbc, CHUNK)]

    # pack edges into per-(block, range) padded slots
    cell_starts = np.zeros(nb * 2 + 1, np.int64)
    np.cumsum(counts.reshape(-1), out=cell_starts[1:])
    pos_in_cell = np.arange(len(srcs)) - cell_starts[cell]
    # flat slot index: block-major [b][r][slot]
    cell_base = np.zeros(nb * 2, np.int64)
    cell_base[0::2] = np.arange(nb) * (cap[0] + cap[1])
    cell_base[1::2] = cell_base[0::2] + cap[0]
    flat = cell_base[cell] + pos_in_cell

    tot = nb * (cap[0] + cap[1])
    idx_all = np.zeros(tot, np.int32)  # padding idx = 0 (valid row at base)
    dstl_all = np.full(tot, 255.0, np.float32)  # padding -> no one-hot match
    idx_all[flat] = (srcs - np.array(bases)[rngs]).astype(np.int32)
    dstl_all[flat] = (dsts & (P - 1)).astype(np.float32)

    iota = np.broadcast_to(np.arange(P, dtype=bf16), (P, P)).copy()
    wt = np.ascontiguousarray(np.asarray(weight, np.float32).T)
    wiht = np.ascontiguousarray(np.asarray(w_ih, np.float32).T)
    bsum = (
        (np.asarray(b_ih, np.float32) + np.asarray(b_hh, np.float32))
        .reshape(4, C)
        .T.copy()
    )

    per_blk = cap[0] + cap[1]
    in_maps = []
    for m in range(N_CORES):
        lo_b = m * nbc
        seg = slice(lo_b * per_blk, (lo_b + nbc) * per_blk)
        idx_c = idx_all[seg].reshape(nbc, per_blk)
        dstl_c = dstl_all[seg].reshape(nbc, per_blk)

        # build per-(chunk, range) sections: [sections] each a flat idx list
        gidx_secs = []
        gdstl_secs = []
        b0 = 0
        for cs in chunk_sizes:
            for r in range(2):
                off = 0 if r == 0 else cap[0]
                sec_idx = idx_c[b0 : b0 + cs, off : off + cap[r]].reshape(-1).copy()
                sec_dstl = (
                    dstl_c[b0 : b0 + cs, off : off + cap[r]].reshape(-1).copy()
                )
                # the gather ucode DROPS a trailing negative index, so the
                # final slot of every call must be >= 0: swap offending real
                # edges with a padding slot of the SAME (block,range) cell.
                sec_tiles = cs * t_r[r]
                call_ts = [CALL_T] * (sec_tiles // CALL_T)
                if sec_tiles % CALL_T:
                    call_ts.append(sec_tiles % CALL_T)
                ends = np.cumsum(np.array(call_ts)) * P - 1  # call-final slots
                end_set = set(int(e) for e in ends)
                for s in ends:
                    s = int(s)
                    if sec_idx[s] >= 0:
                        continue
                    k = s // cap[r]  # cell (block) within section
                    cnt = int(counts[lo_b + b0 + k, r])
                    for p in range(k * cap[r] + cnt, (k + 1) * cap[r]):
                        if p not in end_set:
                            for arr in (sec_idx, sec_dstl):
                                arr[s], arr[p] = arr[p], arr[s]
                            break
                    else:
                        raise RuntimeError("no swap slot for call-final pad")
                gidx_secs.append(sec_idx)
                gdstl_secs.append(sec_dstl)
            b0 += cs

        # idx wrap per CALL: i -> [i%16, i//16], replicated x8 across
        # partitions; concatenate calls/sections along free dim
        def wrap16(a):
            w = np.ascontiguousarray(a.reshape(-1, 16).T.astype(np.int16))
            return np.tile(w, (8, 1))  # [128, len/16]

        gidx_cols = []
        for sec in gidx_secs:
            st = len(sec) // P
            for c0 in range(0, st, CALL_T):
                ct = min(CALL_T, st - c0)
                gidx_cols.append(wrap16(sec[c0 * P : (c0 + ct) * P]))
        gidx = np.concatenate(gidx_cols, axis=1)

        # per-tile metadata, partition-major per section: [128, tiles]
        def tilemeta(secs, dt):
            cols = [s.reshape(-1, P).T for s in secs]  # [128, tiles_sec]
            return np.ascontiguousarray(np.concatenate(cols, axis=1).astype(dt))

        gdstl = tilemeta(gdstl_secs, bf16)

        d1_c = d1[lo_b * P : (lo_b + nbc) * P].reshape(nbc, P).T.copy()  # [128, nbc]

        in_maps.append(
            dict(
                xsrc=ypad,
                xself=np.ascontiguousarray(ypad[lo_b * P : (lo_b + nbc) * P]),
                gidx=np.ascontiguousarray(gidx),
                gdstl=gdstl,
                d1=d1_c,
                iota=iota,
                wt=wt,
                wiht=wiht,
                bias=bsum,
            )
        )
    meta = (t_r, nbc, chunk_sizes, bases, npad)
    return in_maps, meta


def build_program(meta, reps=1, variant="full", nq=1):
    import concourse.bacc as bacc
    import concourse.mybir as mybir
    import concourse.tile as tile
    from concourse.masks import make_identity

    t_r, nbc, chunk_sizes, bases, npad = meta
    f32 = mybir.dt.float32
    bf16 = mybir.dt.bfloat16
    i16 = mybir.dt.int16
    AF = mybir.ActivationFunctionType
    ALU = mybir.AluOpType

    n_tiles = nbc * (t_r[0] + t_r[1])  # edge tiles per core
    idx_w = n_tiles * P // 16  # gidx free dim

    nc = bacc.Bacc("TRN2", num_swdge_queues=nq)
    xsrc = nc.declare_dram_parameter("xsrc", [npad, C], bf16, isOutput=False)
    gidx = nc.declare_dram_parameter("gidx", [P, idx_w], i16, isOutput=False)
    gdstl = nc.declare_dram_parameter("gdstl", [P, n_tiles], bf16, isOutput=False)
    d1 = nc.declare_dram_parameter("d1", [P, nbc], f32, isOutput=False)
    iota = nc.declare_dram_parameter("iota", [P, P], bf16, isOutput=False)
    wt = nc.declare_dram_parameter("wt", [P, P], f32, isOutput=False)
    wiht = nc.declare_dram_parameter("wiht", [P, 4 * C], f32, isOutput=False)
    bias = nc.declare_dram_parameter("bias", [P, 4], f32, isOutput=False)
    out = nc.declare_dram_parameter("out", [nbc * P, C], f32, isOutput=True)
    # per-core slice of the prescaled rows (this core's own dst blocks)
    xself = nc.declare_dram_parameter("xself", [nbc * P, C], bf16, isOutput=False)

    with tile.TileContext(nc) as tc:
        with (
            tc.tile_pool(name="const", bufs=1) as constp,
            tc.tile_pool(name="stag", bufs=2) as stagp,
            tc.tile_pool(name="meta", bufs=2) as metap,
            tc.tile_pool(name="work", bufs=4) as workp,
            tc.tile_pool(name="selfx", bufs=3) as selfp,
            tc.tile_pool(name="psA", bufs=CHUNK, space="PSUM") as psA,
            tc.tile_pool(name="psB", bufs=1, space="PSUM") as psB,
            tc.tile_pool(name="osb", bufs=3) as osbp,
        ):
            iota_sb = constp.tile([P, P], bf16, tag="iota")
            nc.sync.dma_start(out=iota_sb[:], in_=iota[:])
            wt_sb = constp.tile([P, P], f32, tag="wt")
            nc.sync.dma_start(out=wt_sb[:], in_=wt[:])
            wiht_sb = constp.tile([P, 4 * C], f32, tag="wiht")
            nc.sync.dma_start(out=wiht_sb[:], in_=wiht[:])
            bias_sb = constp.tile([P, 4], f32, tag="bias")
            nc.sync.dma_start(out=bias_sb[:], in_=bias[:])
            d1_sb = constp.tile([P, nbc], f32, tag="d1")
            nc.sync.dma_start(out=d1_sb[:], in_=d1[:])
            ident = constp.tile([P, P], f32, tag="ident")
            make_identity(nc, ident[:])
            identb = constp.tile([P, P], bf16, tag="identb")
            nc.vector.tensor_copy(out=identb[:], in_=ident[:])

            # --- LSTM single step -> evolved weight w_new (bf16) ---
            gate_sb = {}
            for m, func, bcol in ((0, AF.Sigmoid, 0), (2, AF.Tanh, 2), (3, AF.Sigmoid, 3)):
                ps = psB.tile([P, P], f32, tag="psb")
                nc.tensor.matmul(
                    out=ps[:],
                    lhsT=wiht_sb[:, m * P : (m + 1) * P],
                    rhs=wt_sb[:],
                    start=True,
                    stop=True,
                )
                sb = constp.tile([P, P], f32, tag=f"gate{m}")
                nc.scalar.activation(
                    out=sb[:], in_=ps[:], func=func, bias=bias_sb[:, bcol : bcol + 1]
                )
                gate_sb[m] = sb
            cT = constp.tile([P, P], f32, tag="cT")
            nc.vector.tensor_mul(out=cT[:], in0=gate_sb[0][:], in1=gate_sb[2][:])
            tcT = constp.tile([P, P], f32, tag="tcT")
            nc.scalar.activation(out=tcT[:], in_=cT[:], func=AF.Tanh)
            wnT = constp.tile([P, P], f32, tag="wnT")
            nc.vector.tensor_mul(out=wnT[:], in0=gate_sb[3][:], in1=tcT[:])
            wn_ps = psB.tile([P, P], f32, tag="psb")
            nc.tensor.transpose(out=wn_ps[:], in_=wnT[:], identity=ident[:])
            wn_sb = constp.tile([P, P], bf16, tag="wn")
            nc.vector.tensor_copy(out=wn_sb[:], in_=wn_ps[:])

            # --- main: chunks of blocks; self pass + 2 gather passes ---
            def emit_main(_iv=None):
              b0 = 0  # first block of chunk
              s_tile = 0  # global edge-tile cursor
              s_idx = 0  # global gidx column cursor (int16 cols)
              n_call = 0  # gather call counter (queue round-robin)
              for cs in chunk_sizes:
                aggs = [
                    psA.tile([P, P], f32, name=f"agg{i}", tag="agg")
                    for i in range(cs)
                ]
                # self-loop pass: aggT[b] = y_block^T (y already dinv-scaled)
                for i in range(cs):
                    b = b0 + i
                    xs = selfp.tile([P, P], bf16, tag="xself")
                    nc.sync.dma_start(
                        out=xs[:], in_=xself[b * P : (b + 1) * P, :]
                    )
                    nc.tensor.matmul(
                        out=aggs[i][:], lhsT=xs[:], rhs=identb[:],
                        start=True, stop=(variant == "gatheronly"),
                    )
                # gather passes
                for r in range(2):
                    T = t_r[r]
                    sec_tiles = cs * T
                    idx_t = metap.tile([P, sec_tiles * 8], i16, tag="idx")
                    nc.sync.dma_start(
                        out=idx_t[:], in_=gidx[:, s_idx : s_idx + sec_tiles * 8]
                    )
                    dstl_t = metap.tile([P, sec_tiles], bf16, tag="dstl")
                    nc.sync.dma_start(
                        out=dstl_t[:], in_=gdstl[:, s_tile : s_tile + sec_tiles]
                    )
                    stag = stagp.tile([P, sec_tiles * P], bf16, tag="stag")
                    if variant == "contig":
                        # same byte volume as the gather, but one sequential
                        # DMA: isolates the random-access/SWDGE cost
                        nc.sync.dma_start(
                            out=stag[:].rearrange(
                                "p (t c) -> p t c", t=sec_tiles
                            ),
                            in_=xsrc[: sec_tiles * P, :].rearrange(
                                "(t p) c -> p t c", p=P
                            ),
                        )
                    for c0 in range(0, sec_tiles, CALL_T):
                        if variant == "contig":
                            break
                        ct = min(CALL_T, sec_tiles - c0)
                        nc.gpsimd.dma_gather(
                            out_ap=stag[:, c0 * P : (c0 + ct) * P].rearrange(
                                "p (t c) -> p t c", t=ct
                            ),
                            in_ap=xsrc[bases[r] :, :],
                            idxs_ap=idx_t[:, c0 * 8 : (c0 + ct) * 8],
                            num_idxs=ct * P,
                            num_idxs_reg=ct * P,
                            elem_size=P,
                            queue_num=n_call % nq,
                        )
                        n_call += 1
                        if variant == "gatheronly":
                            # consume each call's staged tile cheaply so the
                            # final drain waits on the gather DMAs
                            # (unconsumed gathers wedge teardown)
                            dummy = workp.tile([P, 1], bf16, tag="dummy")
                            nc.vector.tensor_copy(
                                out=dummy[:],
                                in_=stag[:, (c0 + ct) * P - 1 : (c0 + ct) * P],
                            )
                    if variant == "gatheronly":
                        s_tile += sec_tiles
                        s_idx += sec_tiles * 8
                        continue
                    for i in range(cs):
                        # one-hot dst selectors for this block's T tiles in a
                        # single DVE op (broadcast APs)
                        s_oh = workp.tile([P, T * P], bf16, tag="soh")
                        nc.vector.tensor_tensor(
                            out=s_oh[:].rearrange("p (t c) -> p t c", t=T),
                            in0=dstl_t[:, i * T : (i + 1) * T].to_broadcast(
                                [P, T, P]
                            ),
                            in1=iota_sb[:].unsqueeze(1).broadcast_to([P, T, P]),
                            op=ALU.is_equal,
                        )
                        last_of_block = (r == 1)
                        for t in range(T):
                            j = i * T + t  # tile within section
                            nc.tensor.matmul(
                                out=aggs[i][:],
                                lhsT=stag[:, j * P : (j + 1) * P],
                                rhs=s_oh[:, t * P : (t + 1) * P],
                                start=False,
                                stop=(last_of_block and t == T - 1),
                            )
                    s_tile += sec_tiles
                    s_idx += sec_tiles * 8
                # flush chunk
                for i in range(cs):
                    b = b0 + i
                    agg_sb = osbp.tile([P, P], bf16, tag="aggsb")
                    nc.vector.tensor_copy(out=agg_sb[:], in_=aggs[i][:])
                    y_ps = psB.tile([P, P], f32, tag="psb")
                    nc.tensor.matmul(
                        out=y_ps[:], lhsT=agg_sb[:], rhs=wn_sb[:],
                        start=True, stop=True,
                    )
                    y_sb = osbp.tile([P, P], f32, tag="ysb")
                    nc.scalar.activation(
                        out=y_sb[:], in_=y_ps[:], func=AF.Copy,
                        scale=d1_sb[:, b : b + 1],
                    )
                    nc.sync.dma_start(
                        out=out[b * P : (b + 1) * P, :], in_=y_sb[:]
                    )
                b0 += cs

            if reps > 1:
                with tc.For_i(0, reps, 1):
                    emit_main()
            else:
                emit_main()

    nc.finalize()
    return nc


def kernel(**inputs) -> np.ndarray:
    from concourse.bass_utils import run_bass_kernel_spmd

    x = inputs["x"]
    n = x.shape[0]
    in_maps, meta = prep_inputs(
        x,
        inputs["edge_index"],
        inputs["weight"],
        inputs["w_ih"],
        inputs["b_ih"],
        inputs["b_hh"],
        n=n,
    )
    nc = build_program(meta)
    res = run_bass_kernel_spmd(nc, in_maps, list(range(N_CORES)))
    full = np.concatenate([r["out"] for r in res.results], axis=0)
    return np.ascontiguousarray(full[:n])


# revision 19
# speedup vs baseline: 2.5825x; 2.5825x over previous
"""EvolveGCN-O forward pass on 8 Trainium2 NeuronCores (Bass/Tile).

Math (reference):
    w_new = LSTM-evolve(weight; w_ih, b_ih+b_hh)          # [C, C]
    out   = D^-1/2 (A + I) D^-1/2  X  w_new               # [N, C]

v2 strategy (per sharding hint: edges + scatter targets sharded):
  * Factor the norm: with y = dinv * x,  out = diag(dinv) (A+I) y W.
    The per-edge norm multiply disappears; y rows are prescaled on the
    host and stored in BF16 (halves gather bytes, enables full-rate
    bf16 PE matmuls; tolerance is 2e-2, bf16 keeps us ~1e-3).
  * Destination nodes padded to NPAD (multiple of 128*8); 128-node
    blocks; each core owns nbc consecutive blocks, processed in chunks
    of 7 (7 PSUM banks accumulate 7 blocks; the 8th bank is scratch).
  * Self-loop term y[i]: contiguous rows DMA'd and transposed into the
    block's PSUM accumulator via a bf16 identity matmul (start=True).
  * Edges: host sorts by dst block and splits by source range (the
    dma_gather index is a SIGNED int16 offset from the call's base row,
    so one call reaches a 65536-row window -> 2 ranges cover N=100k).
    Every (block, range) segment is padded to a uniform tile count;
    padding slots carry dstl=255 so their one-hot column is all-zero
    (>=1 slack slot so no call ends on a negative index, which the
    ucode would drop).
  * Per edge tile of 128: gpsimd.dma_gather stages bf16 rows y[src]
    (one call per CALL_T tiles); the one-hot dst selectors for a whole
    block's tiles are built in ONE DVE is_equal via broadcast APs; PE
    accumulates aggT += tile^T-routed sums. Per block:
    Y = (aggT^T @ w_new) * dinv[dst], DMA out.
  * w_new computed on-device (3 matmuls + activations), redundantly per
    core. No collectives: block ownership makes outputs disjoint.
"""
import os
import sys

for _p in ("/opt/trn_rl_repo", "/root/.axon_site/_ro/trn_rl_repo"):
    if _p not in sys.path:
        sys.path.append(_p)

import numpy as np

N, C, E = 100000, 128, 1600000  # problem shape (hardcoded per spec)
P = 128
N_CORES = 8
CHUNK = 7  # blocks per PSUM-resident chunk
IDX_WIN = 32768  # int16 signed reach below/above base
CALL_T = 8  # edge tiles per dma_gather call (1024 idx = HW per-call max)


def _cdiv(a, b):
    return -(-a // b)


def prep_inputs(x, edge_index, weight, w_ih, b_ih, b_hh, n=N):
    """Host-side sharding/index prep.

    Returns (in_maps, meta) where meta = (t_r tuple, nbc, chunk_sizes).
    """
    import ml_dtypes

    bf16 = ml_dtypes.bfloat16
    x = np.ascontiguousarray(np.asarray(x, dtype=np.float32))
    ei = np.asarray(edge_index)
    src_e = ei[0].astype(np.int64)
    dst_e = ei[1].astype(np.int64)

    npad = _cdiv(n, P * N_CORES) * P * N_CORES
    nb = npad // P
    nbc = nb // N_CORES

    # degrees include self loops
    deg = (np.bincount(dst_e, minlength=n) + 1).astype(np.float32)
    dinv = (1.0 / np.sqrt(deg)).astype(np.float32)
    d1 = np.zeros(npad, np.float32)
    d1[:n] = dinv

    # prescaled node rows y = dinv * x, bf16
    ypad = np.zeros((npad, C), bf16)
    ypad[:n] = (x * dinv[:, None]).astype(bf16)

    # source ranges
    split = npad // 2
    bases = (max(0, split - IDX_WIN), max(0, npad - IDX_WIN))
    rng_of = (src_e >= split).astype(np.int64)

    # sort edges by (block, range, src): ascending src within each cell
    # makes each gather call sweep HBM addresses monotonically (better
    # bank/row locality than random order)
    blk = dst_e >> 7
    order = np.lexsort((src_e, blk * 2 + rng_of))
    srcs = src_e[order]
    dsts = dst_e[order]
    rngs = rng_of[order]

    # per-(block, range) counts -> uniform tile counts. +4 slack slots so
    # every cell keeps spare padding for the call-final swap below.
    cell = blk[order] * 2 + rngs
    counts = np.bincount(cell, minlength=nb * 2).reshape(nb, 2)
    t_r = tuple(int(_cdiv(int(counts[:, r].max()) + 4, P)) for r in range(2))
    cap = (t_r[0] * P, t_r[1] * P)

    # chunk structure (uniform across cores)
    chunk_sizes = [min(CHUNK, nbc - i) for i in range(0, nbc, CHUNK)]

    # pack edges into per-(block, range) padded slots
    cell_starts = np.zeros(nb * 2 + 1, np.int64)
    np.cumsum(counts.reshape(-1), out=cell_starts[1:])
    pos_in_cell = np.arange(len(srcs)) - cell_starts[cell]
    # flat slot index: block-major [b][r][slot]
    cell_base = np.zeros(nb * 2, np.int64)
    cell_base[0::2] = np.arange(nb) * (cap[0] + cap[1])
    cell_base[1::2] = cell_base[0::2] + cap[0]
    flat = cell_base[cell] + pos_in_cell

    tot = nb * (cap[0] + cap[1])
    idx_all = np.zeros(tot, np.int32)
    dstl_all = np.full(tot, 255.0, np.float32)  # padding -> no one-hot match
    pad_all = np.ones(tot, bool)
    idx_all[flat] = (srcs - np.array(bases)[rngs]).astype(np.int32)
    dstl_all[flat] = (dsts & (P - 1)).astype(np.float32)
    pad_all[flat] = False
    # padding fetches duplicate the cell's last real row (open HBM row)
    # instead of hammering row `base` for every pad slot
    for cidx in range(nb * 2):
        b_, r_ = divmod(cidx, 2)
        start = int(cell_base[cidx])
        cnt = int(counts[b_, r_])
        capr = cap[r_]
        if cnt < capr:
            fill = idx_all[start + cnt - 1] if cnt > 0 else 0
            idx_all[start + cnt : start + capr] = fill

    iota = np.broadcast_to(np.arange(P, dtype=bf16), (P, P)).copy()
    wt = np.ascontiguousarray(np.asarray(weight, np.float32).T)
    wiht = np.ascontiguousarray(np.asarray(w_ih, np.float32).T)
    bsum = (
        (np.asarray(b_ih, np.float32) + np.asarray(b_hh, np.float32))
        .reshape(4, C)
        .T.copy()
    )

    per_blk = cap[0] + cap[1]
    in_maps = []
    for m in range(N_CORES):
        lo_b = m * nbc
        seg = slice(lo_b * per_blk, (lo_b + nbc) * per_blk)
        idx_c = idx_all[seg].reshape(nbc, per_blk)
        dstl_c = dstl_all[seg].reshape(nbc, per_blk)
        pad_c = pad_all[seg].reshape(nbc, per_blk)

        # build per-(chunk, range) sections: [sections] each a flat idx list
        gidx_secs = []
        gdstl_secs = []
        b0 = 0
        for cs in chunk_sizes:
            for r in range(2):
                off = 0 if r == 0 else cap[0]
                sec_idx = idx_c[b0 : b0 + cs, off : off + cap[r]].reshape(-1).copy()
                sec_dstl = (
                    dstl_c[b0 : b0 + cs, off : off + cap[r]].reshape(-1).copy()
                )
                sec_pad = pad_c[b0 : b0 + cs, off : off + cap[r]].reshape(-1).copy()
                # the gather ucode DROPS trailing negative indices, so the
                # final slot of every call must be a non-negative REAL edge
                # or padding: swap offending real edges with a padding slot
                # of the SAME (block,range) cell.
                sec_tiles = cs * t_r[r]
                call_ts = [CALL_T] * (sec_tiles // CALL_T)
                if sec_tiles % CALL_T:
                    call_ts.append(sec_tiles % CALL_T)
                ends = np.cumsum(np.array(call_ts)) * P - 1  # call-final slots
                end_set = set(int(e) for e in ends)
                for s in ends:
                    s = int(s)
                    if sec_idx[s] >= 0 or sec_pad[s]:
                        continue
                    k = s // cap[r]  # cell (block) within section
                    cnt = int(counts[lo_b + b0 + k, r])
                    for p in range(k * cap[r] + cnt, (k + 1) * cap[r]):
                        if p not in end_set:
                            for arr in (sec_idx, sec_dstl, sec_pad):
                                arr[s], arr[p] = arr[p], arr[s]
                            break
                    else:
                        raise RuntimeError("no swap slot for call-final pad")
                # trailing padding at each call's end -> negative idx: the
                # ucode skips those fetches entirely
                if not os.environ.get("NO_PAD_DROP"):
                    s0 = 0
                    for ct in call_ts:
                        e = s0 + ct * P - 1
                        j = e
                        while j >= s0 and sec_pad[j]:
                            sec_idx[j] = -1
                            j -= 1
                        s0 = e + 1
                gidx_secs.append(sec_idx)
                gdstl_secs.append(sec_dstl)
            b0 += cs

        # idx wrap per CALL: i -> [i%16, i//16], replicated x8 across
        # partitions; concatenate calls/sections along free dim
        def wrap16(a):
            w = np.ascontiguousarray(a.reshape(-1, 16).T.astype(np.int16))
            return np.tile(w, (8, 1))  # [128, len/16]

        gidx_cols = []
        for sec in gidx_secs:
            st = len(sec) // P
            for c0 in range(0, st, CALL_T):
                ct = min(CALL_T, st - c0)
                gidx_cols.append(wrap16(sec[c0 * P : (c0 + ct) * P]))
        gidx = np.concatenate(gidx_cols, axis=1)

        # per-tile metadata, partition-major per section: [128, tiles]
        def tilemeta(secs, dt):
            cols = [s.reshape(-1, P).T for s in secs]  # [128, tiles_sec]
            return np.ascontiguousarray(np.concatenate(cols, axis=1).astype(dt))

        gdstl = tilemeta(gdstl_secs, bf16)

        d1_c = d1[lo_b * P : (lo_b + nbc) * P].reshape(nbc, P).T.copy()  # [128, nbc]

        in_maps.append(
            dict(
                xsrc=ypad,
                xself=np.ascontiguousarray(ypad[lo_b * P : (lo_b + nbc) * P]),
                gidx=np.ascontiguousarray(gidx),
                gdstl=gdstl,
                d1=d1_c,
                iota=iota,
                wt=wt,
                wiht=wiht,
                bias=bsum,
            )
        )
    meta = (t_r, nbc, chunk_sizes, bases, npad)
    return in_maps, meta


def build_program(meta, reps=1, variant="full", nq=4):
    import concourse.bacc as bacc
    import concourse.mybir as mybir
    import concourse.tile as tile
    from concourse.masks import make_identity

    t_r, nbc, chunk_sizes, bases, npad = meta
    f32 = mybir.dt.float32
    bf16 = mybir.dt.bfloat16
    i16 = mybir.dt.int16
    AF = mybir.ActivationFunctionType
    ALU = mybir.AluOpType

    n_tiles = nbc * (t_r[0] + t_r[1])  # edge tiles per core
    idx_w = n_tiles * P // 16  # gidx free dim

    nc = bacc.Bacc("TRN2", num_swdge_queues=nq)
    xsrc = nc.declare_dram_parameter("xsrc", [npad, C], bf16, isOutput=False)
    gidx = nc.declare_dram_parameter("gidx", [P, idx_w], i16, isOutput=False)
    gdstl = nc.declare_dram_parameter("gdstl", [P, n_tiles], bf16, isOutput=False)
    d1 = nc.declare_dram_parameter("d1", [P, nbc], f32, isOutput=False)
    iota = nc.declare_dram_parameter("iota", [P, P], bf16, isOutput=False)
    wt = nc.declare_dram_parameter("wt", [P, P], f32, isOutput=False)
    wiht = nc.declare_dram_parameter("wiht", [P, 4 * C], f32, isOutput=False)
    bias = nc.declare_dram_parameter("bias", [P, 4], f32, isOutput=False)
    out = nc.declare_dram_parameter("out", [nbc * P, C], f32, isOutput=True)
    # per-core slice of the prescaled rows (this core's own dst blocks)
    xself = nc.declare_dram_parameter("xself", [nbc * P, C], bf16, isOutput=False)

    with tile.TileContext(nc) as tc:
        with (
            tc.tile_pool(name="const", bufs=1) as constp,
            tc.tile_pool(name="stag", bufs=2) as stagp,
            tc.tile_pool(name="meta", bufs=2) as metap,
            tc.tile_pool(name="work", bufs=4) as workp,
            tc.tile_pool(name="selfx", bufs=3) as selfp,
            tc.tile_pool(name="psA", bufs=CHUNK, space="PSUM") as psA,
            tc.tile_pool(name="psB", bufs=1, space="PSUM") as psB,
            tc.tile_pool(name="osb", bufs=3) as osbp,
        ):
            iota_sb = constp.tile([P, P], bf16, tag="iota")
            nc.sync.dma_start(out=iota_sb[:], in_=iota[:])
            wt_sb = constp.tile([P, P], f32, tag="wt")
            nc.sync.dma_start(out=wt_sb[:], in_=wt[:])
            wiht_sb = constp.tile([P, 4 * C], f32, tag="wiht")
            nc.sync.dma_start(out=wiht_sb[:], in_=wiht[:])
            bias_sb = constp.tile([P, 4], f32, tag="bias")
            nc.sync.dma_start(out=bias_sb[:], in_=bias[:])
            d1_sb = constp.tile([P, nbc], f32, tag="d1")
            nc.sync.dma_start(out=d1_sb[:], in_=d1[:])
            ident = constp.tile([P, P], f32, tag="ident")
            make_identity(nc, ident[:])
            identb = constp.tile([P, P], bf16, tag="identb")
            nc.vector.tensor_copy(out=identb[:], in_=ident[:])

            # --- LSTM single step -> evolved weight w_new (bf16) ---
            gate_sb = {}
            for m, func, bcol in ((0, AF.Sigmoid, 0), (2, AF.Tanh, 2), (3, AF.Sigmoid, 3)):
                ps = psB.tile([P, P], f32, tag="psb")
                nc.tensor.matmul(
                    out=ps[:],
                    lhsT=wiht_sb[:, m * P : (m + 1) * P],
                    rhs=wt_sb[:],
                    start=True,
                    stop=True,
                )
                sb = constp.tile([P, P], f32, tag=f"gate{m}")
                nc.scalar.activation(
                    out=sb[:], in_=ps[:], func=func, bias=bias_sb[:, bcol : bcol + 1]
                )
                gate_sb[m] = sb
            cT = constp.tile([P, P], f32, tag="cT")
            nc.vector.tensor_mul(out=cT[:], in0=gate_sb[0][:], in1=gate_sb[2][:])
            tcT = constp.tile([P, P], f32, tag="tcT")
            nc.scalar.activation(out=tcT[:], in_=cT[:], func=AF.Tanh)
            wnT = constp.tile([P, P], f32, tag="wnT")
            nc.vector.tensor_mul(out=wnT[:], in0=gate_sb[3][:], in1=tcT[:])
            wn_ps = psB.tile([P, P], f32, tag="psb")
            nc.tensor.transpose(out=wn_ps[:], in_=wnT[:], identity=ident[:])
            wn_sb = constp.tile([P, P], bf16, tag="wn")
            nc.vector.tensor_copy(out=wn_sb[:], in_=wn_ps[:])

            # --- main: chunks of blocks; self pass + 2 gather passes ---
            def emit_main(_iv=None):
              b0 = 0  # first block of chunk
              s_tile = 0  # global edge-tile cursor
              s_idx = 0  # global gidx column cursor (int16 cols)
              n_call = 0  # gather call counter (queue round-robin)
              for cs in chunk_sizes:
                aggs = [
                    psA.tile([P, P], f32, name=f"agg{i}", tag="agg")
                    for i in range(cs)
                ]
                # self-loop pass: aggT[b] = y_block^T (y already dinv-scaled)
                for i in range(cs):
                    b = b0 + i
                    xs = selfp.tile([P, P], bf16, tag="xself")
                    nc.sync.dma_start(
                        out=xs[:], in_=xself[b * P : (b + 1) * P, :]
                    )
                    nc.tensor.matmul(
                        out=aggs[i][:], lhsT=xs[:], rhs=identb[:],
                        start=True, stop=(variant == "gatheronly"),
                    )
                # gather passes
                for r in range(2):
                    T = t_r[r]
                    sec_tiles = cs * T
                    idx_t = metap.tile([P, sec_tiles * 8], i16, tag="idx")
                    nc.sync.dma_start(
                        out=idx_t[:], in_=gidx[:, s_idx : s_idx + sec_tiles * 8]
                    )
                    dstl_t = metap.tile([P, sec_tiles], bf16, tag="dstl")
                    nc.sync.dma_start(
                        out=dstl_t[:], in_=gdstl[:, s_tile : s_tile + sec_tiles]
                    )
                    stag = stagp.tile([P, sec_tiles * P], bf16, tag="stag")
                    if variant == "contig":
                        # same byte volume as the gather, but one sequential
                        # DMA: isolates the random-access/SWDGE cost
                        nc.sync.dma_start(
                            out=stag[:].rearrange(
                                "p (t c) -> p t c", t=sec_tiles
                            ),
                            in_=xsrc[: sec_tiles * P, :].rearrange(
                                "(t p) c -> p t c", p=P
                            ),
                        )
                    for c0 in range(0, sec_tiles, CALL_T):
                        if variant == "contig":
                            break
                        ct = min(CALL_T, sec_tiles - c0)
                        nc.gpsimd.dma_gather(
                            out_ap=stag[:, c0 * P : (c0 + ct) * P].rearrange(
                                "p (t c) -> p t c", t=ct
                            ),
                            in_ap=xsrc[bases[r] :, :],
                            idxs_ap=idx_t[:, c0 * 8 : (c0 + ct) * 8],
                            num_idxs=ct * P,
                            num_idxs_reg=ct * P,
                            elem_size=P,
                            queue_num=n_call % nq,
                        )
                        n_call += 1
                        if variant == "gatheronly":
                            # consume each call's staged tile cheaply so the
                            # final drain waits on the gather DMAs
                            # (unconsumed gathers wedge teardown)
                            dummy = workp.tile([P, 1], bf16, tag="dummy")
                            nc.vector.tensor_copy(
                                out=dummy[:],
                                in_=stag[:, (c0 + ct) * P - 1 : (c0 + ct) * P],
                            )
                    if variant == "gatheronly":
                        s_tile += sec_tiles
                        s_idx += sec_tiles * 8
                        continue
                    for i in range(cs):
                        # one-hot dst selectors for this block's T tiles in a
                        # single DVE op (broadcast APs)
                        s_oh = workp.tile([P, T * P], bf16, tag="soh")
                        if variant == "nobatch":
                            for t in range(T):
                                nc.vector.tensor_tensor(
                                    out=s_oh[:, t * P : (t + 1) * P],
                                    in0=dstl_t[
                                        :, i * T + t : i * T + t + 1
                                    ].to_broadcast([P, P]),
                                    in1=iota_sb[:],
                                    op=ALU.is_equal,
                                )
                        else:
                            nc.vector.tensor_tensor(
                                out=s_oh[:].rearrange("p (t c) -> p t c", t=T),
                                in0=dstl_t[:, i * T : (i + 1) * T].to_broadcast(
                                    [P, T, P]
                                ),
                                in1=iota_sb[:].unsqueeze(1).broadcast_to(
                                    [P, T, P]
                                ),
                                op=ALU.is_equal,
                            )
                        last_of_block = (r == 1)
                        for t in range(T):
                            j = i * T + t  # tile within section
                            nc.tensor.matmul(
                                out=aggs[i][:],
                                lhsT=stag[:, j * P : (j + 1) * P],
                                rhs=s_oh[:, t * P : (t + 1) * P],
                                start=False,
                                stop=(last_of_block and t == T - 1),
                            )
                    s_tile += sec_tiles
                    s_idx += sec_tiles * 8
                # flush chunk
                for i in range(cs):
                    b = b0 + i
                    agg_sb = osbp.tile([P, P], bf16, tag="aggsb")
                    nc.vector.tensor_copy(out=agg_sb[:], in_=aggs[i][:])
                    y_ps = psB.tile([P, P], f32, tag="psb")
                    nc.tensor.matmul(
                        out=y_ps[:], lhsT=agg_sb[:], rhs=wn_sb[:],
                        start=True, stop=True,
                    )
                    y_sb = osbp.tile([P, P], f32, tag="ysb")
                    nc.scalar.activation(
                        out=y_sb[:], in_=y_ps[:], func=AF.Copy,
                        scale=d1_sb[:, b : b + 1],
                    )
                    nc.sync.dma_start(
                        out=out[b * P : (b + 1) * P, :], in_=y_sb[:]
                    )
                b0 += cs

            if reps > 1:
                with tc.For_i(0, reps, 1):
                    emit_main()
            else:
                emit_main()

    nc.finalize()
    return nc


def kernel(**inputs) -> np.ndarray:
    from concourse.bass_utils import run_bass_kernel_spmd

    x = inputs["x"]
    n = x.shape[0]
    in_maps, meta = prep_inputs(
        x,
        inputs["edge_index"],
        inputs["weight"],
        inputs["w_ih"],
        inputs["b_ih"],
        inputs["b_hh"],
        n=n,
    )
    nc = build_program(meta)
    res = run_bass_kernel_spmd(nc, in_maps, list(range(N_CORES)))
    full = np.concatenate([r["out"] for r in res.results], axis=0)
    return np.ascontiguousarray(full[:n])


# revision 24
# speedup vs baseline: 2.8732x; 1.1125x over previous
"""EvolveGCN-O forward pass on 8 Trainium2 NeuronCores (Bass/Tile).

Math (reference):
    w_new = LSTM-evolve(weight; w_ih, b_ih+b_hh)          # [C, C]
    out   = D^-1/2 (A + I) D^-1/2  X  w_new               # [N, C]

v2 strategy (per sharding hint: edges + scatter targets sharded):
  * Factor the norm: with y = dinv * x,  out = diag(dinv) (A+I) y W.
    The per-edge norm multiply disappears; y rows are prescaled on the
    host and stored in BF16 (halves gather bytes, enables full-rate
    bf16 PE matmuls; tolerance is 2e-2, bf16 keeps us ~1e-3).
  * Destination nodes padded to NPAD (multiple of 128*8); 128-node
    blocks; each core owns nbc consecutive blocks, processed in chunks
    of 7 (7 PSUM banks accumulate 7 blocks; the 8th bank is scratch).
  * Self-loop term y[i]: contiguous rows DMA'd and transposed into the
    block's PSUM accumulator via a bf16 identity matmul (start=True).
  * Edges: host sorts by dst block and splits by source range (the
    dma_gather index is a SIGNED int16 offset from the call's base row,
    so one call reaches a 65536-row window -> 2 ranges cover N=100k).
    Every (block, range) segment is padded to a uniform tile count;
    padding slots carry dstl=255 so their one-hot column is all-zero
    (>=1 slack slot so no call ends on a negative index, which the
    ucode would drop).
  * Per edge tile of 128: gpsimd.dma_gather stages bf16 rows y[src]
    (one call per CALL_T tiles); the one-hot dst selectors for a whole
    block's tiles are built in ONE DVE is_equal via broadcast APs; PE
    accumulates aggT += tile^T-routed sums. Per block:
    Y = (aggT^T @ w_new) * dinv[dst], DMA out.
  * w_new computed on-device (3 matmuls + activations), redundantly per
    core. No collectives: block ownership makes outputs disjoint.
"""
import os
import sys

for _p in ("/opt/trn_rl_repo", "/root/.axon_site/_ro/trn_rl_repo"):
    if _p not in sys.path:
        sys.path.append(_p)

import numpy as np

N, C, E = 100000, 128, 1600000  # problem shape (hardcoded per spec)
P = 128
N_CORES = 8
CHUNK = 7  # blocks per PSUM-resident chunk
IDX_WIN = 32768  # int16 signed reach below/above base
CALL_T = int(os.environ.get("CALL_T", "8"))  # edge tiles per dma_gather call


def _cdiv(a, b):
    return -(-a // b)


def prep_inputs(x, edge_index, weight, w_ih, b_ih, b_hh, n=N):
    """Host-side sharding/index prep.

    Returns (in_maps, meta) where meta = (t_r tuple, nbc, chunk_sizes).
    """
    import ml_dtypes

    bf16 = ml_dtypes.bfloat16
    x = np.ascontiguousarray(np.asarray(x, dtype=np.float32))
    ei = np.asarray(edge_index)
    src_e = ei[0].astype(np.int64)
    dst_e = ei[1].astype(np.int64)

    npad = _cdiv(n, P * N_CORES) * P * N_CORES
    nb = npad // P
    nbc = nb // N_CORES

    # degrees include self loops
    deg = (np.bincount(dst_e, minlength=n) + 1).astype(np.float32)
    dinv = (1.0 / np.sqrt(deg)).astype(np.float32)
    d1 = np.zeros(npad, np.float32)
    d1[:n] = dinv

    # prescaled node rows y = dinv * x, bf16
    ypad = np.zeros((npad, C), bf16)
    ypad[:n] = (x * dinv[:, None]).astype(bf16)

    # source ranges
    split = npad // 2
    bases = (max(0, split - IDX_WIN), max(0, npad - IDX_WIN))
    rng_of = (src_e >= split).astype(np.int64)

    # sort edges by (block, range, src): ascending src within each cell
    # makes each gather call sweep HBM addresses monotonically (better
    # bank/row locality than random order)
    blk = dst_e >> 7
    order = np.lexsort((src_e, blk * 2 + rng_of))
    srcs = src_e[order]
    dsts = dst_e[order]
    rngs = rng_of[order]

    # per-(block, range) counts -> uniform tile counts. +4 slack slots so
    # every cell keeps spare padding for the call-final swap below.
    cell = blk[order] * 2 + rngs
    counts = np.bincount(cell, minlength=nb * 2).reshape(nb, 2)
    t_r = tuple(int(_cdiv(int(counts[:, r].max()) + 4, P)) for r in range(2))
    cap = (t_r[0] * P, t_r[1] * P)

    # chunk structure (uniform across cores)
    chunk_sizes = [min(CHUNK, nbc - i) for i in range(0, nbc, CHUNK)]

    # pack edges into per-(block, range) padded slots
    cell_starts = np.zeros(nb * 2 + 1, np.int64)
    np.cumsum(counts.reshape(-1), out=cell_starts[1:])
    pos_in_cell = np.arange(len(srcs)) - cell_starts[cell]
    # flat slot index: block-major [b][r][slot]
    cell_base = np.zeros(nb * 2, np.int64)
    cell_base[0::2] = np.arange(nb) * (cap[0] + cap[1])
    cell_base[1::2] = cell_base[0::2] + cap[0]
    flat = cell_base[cell] + pos_in_cell

    tot = nb * (cap[0] + cap[1])
    idx_all = np.zeros(tot, np.int32)
    dstl_all = np.full(tot, 255.0, np.float32)  # padding -> no one-hot match
    pad_all = np.ones(tot, bool)
    idx_all[flat] = (srcs - np.array(bases)[rngs]).astype(np.int32)
    dstl_all[flat] = (dsts & (P - 1)).astype(np.float32)
    pad_all[flat] = False
    # padding fetches duplicate the cell's last real row (open HBM row)
    # instead of hammering row `base` for every pad slot
    for cidx in range(nb * 2):
        b_, r_ = divmod(cidx, 2)
        start = int(cell_base[cidx])
        cnt = int(counts[b_, r_])
        capr = cap[r_]
        if cnt < capr:
            fill = idx_all[start + cnt - 1] if cnt > 0 else 0
            idx_all[start + cnt : start + capr] = fill

    iota = np.broadcast_to(np.arange(P, dtype=bf16), (P, P)).copy()
    wt = np.ascontiguousarray(np.asarray(weight, np.float32).T)
    wiht = np.ascontiguousarray(np.asarray(w_ih, np.float32).T)
    bsum = (
        (np.asarray(b_ih, np.float32) + np.asarray(b_hh, np.float32))
        .reshape(4, C)
        .T.copy()
    )

    per_blk = cap[0] + cap[1]
    in_maps = []
    for m in range(N_CORES):
        lo_b = m * nbc
        seg = slice(lo_b * per_blk, (lo_b + nbc) * per_blk)
        idx_c = idx_all[seg].reshape(nbc, per_blk)
        dstl_c = dstl_all[seg].reshape(nbc, per_blk)
        pad_c = pad_all[seg].reshape(nbc, per_blk)

        # build per-(chunk, range) sections: [sections] each a flat idx list
        gidx_secs = []
        gdstl_secs = []
        b0 = 0
        for cs in chunk_sizes:
            for r in range(2):
                off = 0 if r == 0 else cap[0]
                sec_idx = idx_c[b0 : b0 + cs, off : off + cap[r]].reshape(-1).copy()
                sec_dstl = (
                    dstl_c[b0 : b0 + cs, off : off + cap[r]].reshape(-1).copy()
                )
                sec_pad = pad_c[b0 : b0 + cs, off : off + cap[r]].reshape(-1).copy()
                # the gather ucode DROPS trailing negative indices, so the
                # final slot of every call must be a non-negative REAL edge
                # or padding: swap offending real edges with a padding slot
                # of the SAME (block,range) cell.
                sec_tiles = cs * t_r[r]
                call_ts = [CALL_T] * (sec_tiles // CALL_T)
                if sec_tiles % CALL_T:
                    call_ts.append(sec_tiles % CALL_T)
                ends = np.cumsum(np.array(call_ts)) * P - 1  # call-final slots
                end_set = set(int(e) for e in ends)
                for s in ends:
                    s = int(s)
                    if sec_idx[s] >= 0 or sec_pad[s]:
                        continue
                    k = s // cap[r]  # cell (block) within section
                    cnt = int(counts[lo_b + b0 + k, r])
                    for p in range(k * cap[r] + cnt, (k + 1) * cap[r]):
                        if p not in end_set:
                            for arr in (sec_idx, sec_dstl, sec_pad):
                                arr[s], arr[p] = arr[p], arr[s]
                            break
                    else:
                        raise RuntimeError("no swap slot for call-final pad")
                # trailing padding at each call's end -> negative idx makes
                # the ucode skip those fetches, but the decode-side ring
                # bookkeeping then desyncs (reserves for the static count)
                # and wedges the device -> keep disabled
                if os.environ.get("PAD_DROP"):
                    s0 = 0
                    for ct in call_ts:
                        e = s0 + ct * P - 1
                        j = e
                        while j >= s0 and sec_pad[j]:
                            sec_idx[j] = -1
                            j -= 1
                        s0 = e + 1
                gidx_secs.append(sec_idx)
                gdstl_secs.append(sec_dstl)
            b0 += cs

        # idx wrap per CALL: i -> [i%16, i//16], replicated x8 across
        # partitions; concatenate calls/sections along free dim
        def wrap16(a):
            w = np.ascontiguousarray(a.reshape(-1, 16).T.astype(np.int16))
            return np.tile(w, (8, 1))  # [128, len/16]

        gidx_cols = []
        for sec in gidx_secs:
            st = len(sec) // P
            for c0 in range(0, st, CALL_T):
                ct = min(CALL_T, st - c0)
                gidx_cols.append(wrap16(sec[c0 * P : (c0 + ct) * P]))
        gidx = np.concatenate(gidx_cols, axis=1)

        # per-tile metadata, partition-major per section: [128, tiles]
        def tilemeta(secs, dt):
            cols = [s.reshape(-1, P).T for s in secs]  # [128, tiles_sec]
            return np.ascontiguousarray(np.concatenate(cols, axis=1).astype(dt))

        gdstl = tilemeta(gdstl_secs, bf16)

        d1_c = d1[lo_b * P : (lo_b + nbc) * P].reshape(nbc, P).T.copy()  # [128, nbc]

        in_maps.append(
            dict(
                xsrc=ypad,
                xself=np.ascontiguousarray(ypad[lo_b * P : (lo_b + nbc) * P]),
                gidx=np.ascontiguousarray(gidx),
                gdstl=gdstl,
                d1=d1_c,
                iota=iota,
                wt=wt,
                wiht=wiht,
                bias=bsum,
            )
        )
    meta = (t_r, nbc, chunk_sizes, bases, npad)
    return in_maps, meta


def build_program(meta, reps=1, variant="full", nq=4):
    import concourse.bacc as bacc
    import concourse.mybir as mybir
    import concourse.tile as tile
    from concourse.masks import make_identity

    t_r, nbc, chunk_sizes, bases, npad = meta
    f32 = mybir.dt.float32
    bf16 = mybir.dt.bfloat16
    i16 = mybir.dt.int16
    AF = mybir.ActivationFunctionType
    ALU = mybir.AluOpType

    n_tiles = nbc * (t_r[0] + t_r[1])  # edge tiles per core
    idx_w = n_tiles * P // 16  # gidx free dim

    nc = bacc.Bacc(
        "TRN2",
        num_swdge_queues=nq,
        dynamic_dma_scratch_size=int(os.environ.get("DMA_SCRATCH", "16384")),
    )
    xsrc = nc.declare_dram_parameter("xsrc", [npad, C], bf16, isOutput=False)
    gidx = nc.declare_dram_parameter("gidx", [P, idx_w], i16, isOutput=False)
    gdstl = nc.declare_dram_parameter("gdstl", [P, n_tiles], bf16, isOutput=False)
    d1 = nc.declare_dram_parameter("d1", [P, nbc], f32, isOutput=False)
    iota = nc.declare_dram_parameter("iota", [P, P], bf16, isOutput=False)
    wt = nc.declare_dram_parameter("wt", [P, P], f32, isOutput=False)
    wiht = nc.declare_dram_parameter("wiht", [P, 4 * C], f32, isOutput=False)
    bias = nc.declare_dram_parameter("bias", [P, 4], f32, isOutput=False)
    out = nc.declare_dram_parameter("out", [nbc * P, C], f32, isOutput=True)
    # per-core slice of the prescaled rows (this core's own dst blocks)
    xself = nc.declare_dram_parameter("xself", [nbc * P, C], bf16, isOutput=False)

    with tile.TileContext(nc) as tc:
        with (
            tc.tile_pool(name="const", bufs=1) as constp,
            tc.tile_pool(name="stag", bufs=3) as stagp,
            tc.tile_pool(name="meta", bufs=3) as metap,
            tc.tile_pool(name="work", bufs=6) as workp,
            tc.tile_pool(name="selfx", bufs=4) as selfp,
            tc.tile_pool(name="psA", bufs=CHUNK, space="PSUM") as psA,
            tc.tile_pool(name="psB", bufs=1, space="PSUM") as psB,
            tc.tile_pool(name="osb", bufs=4) as osbp,
        ):
            iota_sb = constp.tile([P, P], bf16, tag="iota")
            nc.sync.dma_start(out=iota_sb[:], in_=iota[:])
            wt_sb = constp.tile([P, P], f32, tag="wt")
            nc.sync.dma_start(out=wt_sb[:], in_=wt[:])
            wiht_sb = constp.tile([P, 4 * C], f32, tag="wiht")
            nc.sync.dma_start(out=wiht_sb[:], in_=wiht[:])
            bias_sb = constp.tile([P, 4], f32, tag="bias")
            nc.sync.dma_start(out=bias_sb[:], in_=bias[:])
            d1_sb = constp.tile([P, nbc], f32, tag="d1")
            nc.sync.dma_start(out=d1_sb[:], in_=d1[:])
            ident = constp.tile([P, P], f32, tag="ident")
            make_identity(nc, ident[:])
            identb = constp.tile([P, P], bf16, tag="identb")
            nc.vector.tensor_copy(out=identb[:], in_=ident[:])

            # --- LSTM single step -> evolved weight w_new (bf16) ---
            gate_sb = {}
            for m, func, bcol in ((0, AF.Sigmoid, 0), (2, AF.Tanh, 2), (3, AF.Sigmoid, 3)):
                ps = psB.tile([P, P], f32, tag="psb")
                nc.tensor.matmul(
                    out=ps[:],
                    lhsT=wiht_sb[:, m * P : (m + 1) * P],
                    rhs=wt_sb[:],
                    start=True,
                    stop=True,
                )
                sb = constp.tile([P, P], f32, tag=f"gate{m}")
                nc.scalar.activation(
                    out=sb[:], in_=ps[:], func=func, bias=bias_sb[:, bcol : bcol + 1]
                )
                gate_sb[m] = sb
            cT = constp.tile([P, P], f32, tag="cT")
            nc.vector.tensor_mul(out=cT[:], in0=gate_sb[0][:], in1=gate_sb[2][:])
            tcT = constp.tile([P, P], f32, tag="tcT")
            nc.scalar.activation(out=tcT[:], in_=cT[:], func=AF.Tanh)
            wnT = constp.tile([P, P], f32, tag="wnT")
            nc.vector.tensor_mul(out=wnT[:], in0=gate_sb[3][:], in1=tcT[:])
            wn_ps = psB.tile([P, P], f32, tag="psb")
            nc.tensor.transpose(out=wn_ps[:], in_=wnT[:], identity=ident[:])
            wn_sb = constp.tile([P, P], bf16, tag="wn")
            nc.vector.tensor_copy(out=wn_sb[:], in_=wn_ps[:])

            # --- main: chunks of blocks; self pass + 2 gather passes ---
            def emit_main(_iv=None):
              b0 = 0  # first block of chunk
              s_tile = 0  # global edge-tile cursor
              s_idx = 0  # global gidx column cursor (int16 cols)
              n_call = 0  # gather call counter (queue round-robin)
              for cs in chunk_sizes:
                aggs = [
                    psA.tile([P, P], f32, name=f"agg{i}", tag="agg")
                    for i in range(cs)
                ]
                # self-loop pass: aggT[b] = y_block^T (y already dinv-scaled)
                for i in range(cs):
                    b = b0 + i
                    xs = selfp.tile([P, P], bf16, tag="xself")
                    nc.sync.dma_start(
                        out=xs[:], in_=xself[b * P : (b + 1) * P, :]
                    )
                    nc.tensor.matmul(
                        out=aggs[i][:], lhsT=xs[:], rhs=identb[:],
                        start=True, stop=(variant == "gatheronly"),
                    )
                # gather passes
                for r in range(2):
                    T = t_r[r]
                    sec_tiles = cs * T
                    idx_t = metap.tile([P, sec_tiles * 8], i16, tag="idx")
                    nc.sync.dma_start(
                        out=idx_t[:], in_=gidx[:, s_idx : s_idx + sec_tiles * 8]
                    )
                    dstl_t = metap.tile([P, sec_tiles], bf16, tag="dstl")
                    nc.sync.dma_start(
                        out=dstl_t[:], in_=gdstl[:, s_tile : s_tile + sec_tiles]
                    )
                    stag = stagp.tile([P, sec_tiles * P], bf16, tag="stag")
                    if variant == "contig":
                        # same byte volume as the gather, but one sequential
                        # DMA: isolates the random-access/SWDGE cost
                        nc.sync.dma_start(
                            out=stag[:].rearrange(
                                "p (t c) -> p t c", t=sec_tiles
                            ),
                            in_=xsrc[: sec_tiles * P, :].rearrange(
                                "(t p) c -> p t c", p=P
                            ),
                        )
                    for c0 in range(0, sec_tiles, CALL_T):
                        if variant == "contig":
                            break
                        ct = min(CALL_T, sec_tiles - c0)
                        nc.gpsimd.dma_gather(
                            out_ap=stag[:, c0 * P : (c0 + ct) * P].rearrange(
                                "p (t c) -> p t c", t=ct
                            ),
                            in_ap=xsrc[bases[r] :, :],
                            idxs_ap=idx_t[:, c0 * 8 : (c0 + ct) * 8],
                            num_idxs=ct * P,
                            num_idxs_reg=ct * P,
                            elem_size=P,
                            queue_num=n_call % nq,
                        )
                        n_call += 1
                        if variant == "gatheronly":
                            # consume each call's staged tile cheaply so the
                            # final drain waits on the gather DMAs
                            # (unconsumed gathers wedge teardown)
                            dummy = workp.tile([P, 1], bf16, tag="dummy")
                            nc.vector.tensor_copy(
                                out=dummy[:],
                                in_=stag[:, (c0 + ct) * P - 1 : (c0 + ct) * P],
                            )
                    if variant == "gatheronly":
                        s_tile += sec_tiles
                        s_idx += sec_tiles * 8
                        continue
                    for i in range(cs):
                        # one-hot dst selectors for this block's T tiles in a
                        # single DVE op (broadcast APs)
                        s_oh = workp.tile([P, T * P], bf16, tag="soh")
                        if variant == "nobatch":
                            for t in range(T):
                                nc.vector.tensor_tensor(
                                    out=s_oh[:, t * P : (t + 1) * P],
                                    in0=dstl_t[
                                        :, i * T + t : i * T + t + 1
                                    ].to_broadcast([P, P]),
                                    in1=iota_sb[:],
                                    op=ALU.is_equal,
                                )
                        else:
                            nc.vector.tensor_tensor(
                                out=s_oh[:].rearrange("p (t c) -> p t c", t=T),
                                in0=dstl_t[:, i * T : (i + 1) * T].to_broadcast(
                                    [P, T, P]
                                ),
                                in1=iota_sb[:].unsqueeze(1).broadcast_to(
                                    [P, T, P]
                                ),
                                op=ALU.is_equal,
                            )
                        last_of_block = (r == 1)
                        for t in range(T):
                            j = i * T + t  # tile within section
                            nc.tensor.matmul(
                                out=aggs[i][:],
                                lhsT=stag[:, j * P : (j + 1) * P],
                                rhs=s_oh[:, t * P : (t + 1) * P],
                                start=False,
                                stop=(last_of_block and t == T - 1),
                            )
                    s_tile += sec_tiles
                    s_idx += sec_tiles * 8
                # flush chunk: W-multiply back into each block's OWN psum
                # bank (no shared psB tile -> the 7 flushes pipeline freely);
                # PSUM<->SBUF copies ride the otherwise-idle ACT engine
                for i in range(cs):
                    b = b0 + i
                    agg_sb = osbp.tile([P, P], bf16, tag="aggsb")
                    nc.scalar.activation(
                        out=agg_sb[:], in_=aggs[i][:], func=AF.Copy
                    )
                    nc.tensor.matmul(
                        out=aggs[i][:], lhsT=agg_sb[:], rhs=wn_sb[:],
                        start=True, stop=True,
                    )
                    y_sb = osbp.tile([P, P], f32, tag="ysb")
                    nc.scalar.activation(
                        out=y_sb[:], in_=aggs[i][:], func=AF.Copy,
                        scale=d1_sb[:, b : b + 1],
                    )
                    nc.sync.dma_start(
                        out=out[b * P : (b + 1) * P, :], in_=y_sb[:]
                    )
                b0 += cs

            if reps > 1:
                with tc.For_i(0, reps, 1):
                    emit_main()
            else:
                emit_main()

    nc.finalize()
    return nc


def kernel(**inputs) -> np.ndarray:
    from concourse.bass_utils import run_bass_kernel_spmd

    x = inputs["x"]
    n = x.shape[0]
    in_maps, meta = prep_inputs(
        x,
        inputs["edge_index"],
        inputs["weight"],
        inputs["w_ih"],
        inputs["b_ih"],
        inputs["b_hh"],
        n=n,
    )
    nc = build_program(meta)
    res = run_bass_kernel_spmd(nc, in_maps, list(range(N_CORES)))
    full = np.concatenate([r["out"] for r in res.results], axis=0)
    return np.ascontiguousarray(full[:n])
